# revision 52
# baseline (speedup 1.0000x reference)
"""GQA sparse-attention (sink + sliding window) kernel for 8 TRN2 NeuronCores.

Problem: nn_MultiHeadSelfAttentionModern (B=1, T=2048, D=2048, 32 q heads,
8 KV heads, d_head=64, WINDOW=2048, SINK=64, start_pos=2048, cache_len=2048).

Since S = cache_len + T = 4096 > WINDOW + SINK = 2112, the effective keys are
just kv_cache[:, :, :64] (the sink, used raw for both K and V) plus the 2048
new RoPE'd k (and raw new v).  Sharding: tensor-parallel by KV head — core i
owns KV head i and its 4 query heads, with Wq/Wk/Wv column-sharded and Wo
row-sharded; partial outputs are summed on the host (+ bo).

Design notes, driven by the cost model (PE matmul time = moving rows x
0.42ns; the ACT exp stream is ~145us of serial work; the PE queue is
in-order, so score-tile bursts couple the PE to the exp pace through the
double-buffered scores psum):
  - bf16 datapath end to end (halves DMA; PE cost unchanged; measured
    6.7e-3 rel err vs the 2e-2 gate)
  - k+v projections merged into one 128-wide matmul (cost is per moving
    row, independent of output partition count)
  - RoPE's 32-row swap done on the PE with a host-built permutation matrix
    (SBUF-SBUF swap DMAs each held the global HWDGE ~625ns and serialized
    the rope chain); cos/sin muls on DVE in bf16
  - within each chunk, q23's matmuls are deferred past the kk/q01 ropes so
    the first exp fires ~35 PE matmuls earlier; scores+exp stream as paced
    single-tile thunks (pend/pump in phase 1, per-st feeds inside ctx
    blocks in phase 2) so the ACT engine starts at ~20us and the PE never
    waits on the exp stream; wT triple-buffered, each buffer freed exactly
    one block before reuse
  - v transposes run as feed thunks through scores-psum regions at the
    start of phase 2, off the projection critical chain; small sums/qx
    copies routed through SWDGE (gpsimd) to keep HWDGE free
  - softmax denominators: a ones-column in v_sb makes the ctx matmul
    produce them; reciprocals are broadcast across partitions with a
    selector matmul (bsel: head j*2+h -> partitions h*64..) instead of the
    baseline's DRAM round-trip
  - every constant load is dispatched at the latest point program order
    allows (just before its first consumer is issued), so x quads never
    queue behind constants on the exclusive DMA engines
  - measured (cost-model timeline): 220.0us vs the 289.7us baseline, with
    PE busy ~193us at ~88% occupancy as the binding resource; verified
    rel err 6.7e-3 vs the 2e-2 gate
"""

import numpy as np

T = 2048
DMODEL = 2048
NKV = 8
GROUP = 4
DH = 64
SINK = 64
NST = 17  # s-tiles: 16 full 128-tiles of new tokens + 1 sink tile (64 rows)
SCALE = 0.125  # 1/sqrt(64)

_CACHE = {}


def _build_nc():
    import concourse.bass as bass
    import concourse.mybir as mybir
    import concourse.tile as tile
    from concourse import bacc
    from concourse.masks import make_identity

    f32 = mybir.dt.float32
    f32r = mybir.dt.float32r
    bf16 = mybir.dt.bfloat16

    nc = bacc.Bacc("TRN2", target_bir_lowering=False, debug=False, num_devices=NKV)

    xT = nc.declare_dram_parameter("xT", [DMODEL, T], bf16, isOutput=False).ap()
    wq = nc.declare_dram_parameter("wq", [DMODEL, GROUP * DH], bf16, isOutput=False).ap()
    wkv = nc.declare_dram_parameter("wkv", [DMODEL, 2 * DH], bf16, isOutput=False).ap()
    wo = nc.declare_dram_parameter("wo", [GROUP * DH, DMODEL], bf16, isOutput=False).ap()
    sink_kT = nc.declare_dram_parameter("sink_kT", [DH, SINK], bf16, isOutput=False).ap()
    sink_v = nc.declare_dram_parameter("sink_v", [SINK, DH], bf16, isOutput=False).ap()
    cosb = nc.declare_dram_parameter("cosb", [128, T], bf16, isOutput=False).ap()
    sinb = nc.declare_dram_parameter("sinb", [128, T], bf16, isOutput=False).ap()
    # swap permutation: swp[p, i] = 1 iff p = (i+32 mod 64) within i's 64-block
    swp = nc.declare_dram_parameter("swp", [128, 128], bf16, isOutput=False).ap()
    # denominator broadcast selectors: bsel[h, j*128+p] = 1 iff h = j*2 + p//64
    bsel = nc.declare_dram_parameter("bsel", [GROUP, 256], bf16, isOutput=False).ap()
    out = nc.declare_dram_parameter("out", [T, DMODEL], bf16, isOutput=True).ap()

    with (
        tile.TileContext(nc) as tc,
        tc.tile_pool(name="persist", bufs=1) as persist,
        # scores psum (4 banks double buffered): the exp stream must never
        # wait on projection psum
        tc.tile_pool(name="psA", bufs=2, space="PSUM") as psA,
        tc.tile_pool(name="pm", bufs=1) as pm,
    ):
        # q01/q23: heads (0,1) and (2,3) on partition halves; odd q heads are
        # extracted to base-partition-0 tiles qx1/qx3 post-rope.
        q01 = persist.tile([128, T], bf16, tag="q01")
        q23 = persist.tile([128, T], bf16, tag="q23")
        qx1 = persist.tile([DH, T], bf16, tag="qx1")
        qx3 = persist.tile([DH, T], bf16, tag="qx3")
        kk = persist.tile([DH, T], bf16, tag="kk")
        vTt = persist.tile([DH, T], f32, tag="vTt")
        v_sb = persist.tile([128, NST, DH + 1], bf16, tag="v_sb")
        ctxT = [persist.tile([128, T], bf16, tag=f"ctxT{j}", name=f"ctxT{j}") for j in range(2)]
        ident = persist.tile([SINK, SINK], f32, tag="ident")
        sink_kT_sb = persist.tile([DH, SINK], bf16, tag="sink_kT")
        swp_sb = persist.tile([128, 128], bf16, tag="swp")
        bsel_sb = persist.tile([GROUP, 256], bf16, tag="bsel")
        sums_sb = pm.tile([GROUP, T], f32, tag="sums_sb")

        xTr = xT.rearrange("(k p) t -> p k t", p=128)
        wkvr = wkv.rearrange("(k p) m -> p k m", p=128)
        wqr = wq.rearrange("(k p) m -> p k m", p=128)

        # wT buffers rotate out of pm, triple buffered
        def new_wT():
            return pm.tile([128, NST, 1024], bf16, tag="wT", bufs=3, name="wTm")

        def emit_tile(qt, wT, c0, st):
            """One s-tile of scores (PE) + its exp (ACT).

            The PE queue is in-order and scores psum is double buffered, so
            bursts of more than ~2 of these couple the PE to the ACT exp
            pace; callers must interleave them with other PE work.
            """
            p = SINK if st == NST - 1 else 128
            sps = psA.tile([128, 1024], f32, tag="sps", name="sps")
            if st == NST - 1:
                lhsT = sink_kT_sb
            else:
                lhsT = kk[:, st * 128 : (st + 1) * 128]
            for u in range(2):
                nc.tensor.matmul(
                    sps[0:p, u * 512 : (u + 1) * 512],
                    lhsT=lhsT,
                    rhs=qt[:, c0 + u * 512 : c0 + (u + 1) * 512],
                    start=True,
                    stop=True,
                )
            nc.scalar.activation(
                out=wT[0:p, st, :],
                in_=sps[0:p, :],
                func=mybir.ActivationFunctionType.Exp,
                scale=SCALE,
            )

        pend = []  # queued emission thunks, paced via pump()

        def pump(n):
            for _ in range(min(n, len(pend))):
                pend.pop(0)()

        with (
            tc.tile_pool(name="psB", bufs=1, space="PSUM") as psB,
            tc.tile_pool(name="pw", bufs=1) as pw,
            tc.tile_pool(name="px", bufs=2) as px,
        ):
            # ---- phase 1: projections + per-chunk rope ----
            # first weight piece alone so the k=0 matmuls start at ~1.5us
            # lo/hi halves as separate tiles: reader-after-writer ordering
            # is per tile, so k<8 matmuls issued after the hi-half DMAs must
            # not inherit a dependency on them
            wq_lo = pw.tile([128, 8, GROUP * DH], bf16, tag="wq_lo")
            wq_hi = pw.tile([128, 8, GROUP * DH], bf16, tag="wq_hi")
            wkv_lo = pw.tile([128, 8, 2 * DH], bf16, tag="wkv_lo")
            wkv_hi = pw.tile([128, 8, 2 * DH], bf16, tag="wkv_hi")

            def wqs(k):
                return wq_lo[:, k] if k < 8 else wq_hi[:, k - 8]

            def wkvs(k):
                return wkv_lo[:, k] if k < 8 else wkv_hi[:, k - 8]

            cos_sb = pw.tile([128, T], bf16, tag="cos_sb")
            sin_sb = pw.tile([128, T], bf16, tag="sin_sb")
            sinkv_st = pw.tile([SINK, DH], bf16, tag="sinkv_st")

            def rope_chunk(tgt, cs, pp):
                """tgt[0:pp, cs] <- tgt*C + swap32(tgt)*S on token slice cs."""
                n = cs.stop - cs.start
                aux = psB.tile([128, 512], f32, tag="aux", name="aux")
                nc.tensor.matmul(
                    aux[0:pp, 0:n],
                    lhsT=swp_sb[0:pp, 0:pp],
                    rhs=tgt[0:pp, cs],
                    start=True,
                    stop=True,
                )
                swt = px.tile([128, 512], bf16, tag="sw")
                nc.vector.tensor_mul(swt[0:pp, 0:n], aux[0:pp, 0:n], sin_sb[0:pp, cs])
                nc.vector.tensor_mul(tgt[0:pp, cs], tgt[0:pp, cs], cos_sb[0:pp, cs])
                nc.vector.tensor_add(tgt[0:pp, cs], tgt[0:pp, cs], swt[0:pp, 0:n])

            wT_early = {}
            qsrc = [q01[0:DH, :], qx1, q23[0:DH, :], qx3]

            for c in range(4):  # token chunks of 512
                q01ps = psB.tile([128, 512], f32, tag="q01ps")
                q23ps = psB.tile([128, 512], f32, tag="q23ps")
                kvps = psB.tile([128, 512], f32, tag="kvps")
                cs = slice(c * 512, (c + 1) * 512)
                # kv + q01 matmuls stream per x quad; q23's are deferred past
                # the kk/q01 ropes (they gate the early exp stream), so the
                # first exp fires after ~70 PE matmuls instead of ~105
                xts = []
                for quad in range(4):
                    if c == 0 and quad == 0:
                        # first 2 x pieces + first weight halves alone so the
                        # first projection matmul starts at ~2.5us
                        xa = px.tile([128, 1, 512], bf16, tag="xt0", bufs=2)
                        nc.sync.dma_start(out=xa, in_=xTr[:, 0:1, cs])
                        nc.sync.dma_start(out=wkv_lo[:, 0:2], in_=wkvr[:, 0:2])
                        nc.sync.dma_start(out=wq_lo[:, 0:2], in_=wqr[:, 0:2])
                        xb = px.tile([128, 3, 512], bf16, tag="xt1", bufs=1)
                        nc.sync.dma_start(out=xb, in_=xTr[:, 1:4, cs])
                        nc.sync.dma_start(out=wkv_lo[:, 2:4], in_=wkvr[:, 2:4])
                        nc.sync.dma_start(out=wq_lo[:, 2:4], in_=wqr[:, 2:4])
                        nc.sync.dma_start(out=wkv_lo[:, 4:8], in_=wkvr[:, 4:8])
                        nc.sync.dma_start(out=wq_lo[:, 4:8], in_=wqr[:, 4:8])
                        parts = [(xa, 0, 1), (xb, 1, 3)]
                    else:
                        xt = px.tile([128, 4, 512], bf16, tag="xt", bufs=4)
                        nc.sync.dma_start(
                            out=xt,
                            in_=xTr[:, quad * 4 : (quad + 1) * 4, cs],
                        )
                        parts = [(xt, quad * 4, 4)]
                        # each constant load is dispatched as late as the
                        # program order of its first consumer allows, so x
                        # quads never queue behind it on the DMA engines
                        if c == 0 and quad == 1:
                            # hi weight halves: consumed by this chunk's k=8
                            nc.sync.dma_start(out=wkv_hi, in_=wkvr[:, 8:16])
                            nc.sync.dma_start(out=wq_hi, in_=wqr[:, 8:16])
                        elif c == 0 and quad == 3:
                            # rope tables: consumed right after this quad
                            nc.sync.dma_start(out=swp_sb, in_=swp)
                            nc.sync.dma_start(out=cos_sb, in_=cosb)
                            nc.sync.dma_start(out=sin_sb, in_=sinb)
                        elif c == 1 and quad == 0:
                            # consumed by the first emission / phase 2
                            nc.sync.dma_start(out=sink_kT_sb, in_=sink_kT)
                            nc.sync.dma_start(out=sinkv_st, in_=sink_v)
                            nc.sync.dma_start(out=bsel_sb, in_=bsel)
                    xts.extend(parts)
                    for xtile, k0, kn in parts:
                        for k4 in range(kn):
                            k = k0 + k4
                            nc.tensor.matmul(
                                kvps, lhsT=wkvs(k),
                                rhs=xtile[:, k4, :], start=(k == 0), stop=(k == 15),
                            )
                            nc.tensor.matmul(
                                q01ps, lhsT=wqs(k)[:, 0:128], rhs=xtile[:, k4, :],
                                start=(k == 0), stop=(k == 15),
                            )
                        pump(2)
                nc.vector.tensor_copy(out=kk[:, cs], in_=kvps[0:DH, :])
                nc.vector.tensor_copy(out=q01[:, cs], in_=q01ps)
                nc.vector.tensor_copy(out=vTt[:, cs], in_=kvps[DH:128, :])
                if c == 0:
                    # nothing waits on chunk-0 ropes yet: run q23's matmuls
                    # first so the PE isn't idle behind the copy chain
                    for xtile, k0, kn in xts:
                        for k4 in range(kn):
                            k = k0 + k4
                            nc.tensor.matmul(
                                q23ps, lhsT=wqs(k)[:, 128:256],
                                rhs=xtile[:, k4, :], start=(k == 0), stop=(k == 15),
                            )
                else:
                    pump(2)  # queued scores read already-roped chunks: free
                    # PE work while the kk/q01 copies land
                # rope (k first: it gates all heads' scores); pumped tiles
                # read already-roped chunks, covering the copy/swap latency
                rope_chunk(kk, cs, DH)
                pump(1)
                rope_chunk(q01, cs, 128)
                pump(1)
                # odd head to a base-partition-0 tile (post-rope, per chunk);
                # SWDGE: keeps the contended HWDGE free for x/weight loads
                nc.gpsimd.dma_start(out=qx1[:, cs], in_=q01[DH:128, cs])
                # early exp: queue scores+exp for the chunk-0 blocks of heads
                # 0/1 as soon as the needed k/q chunks are roped; pump() paces
                # them between projection quads so the ACT engine (the ~145us
                # serial backbone) starts at ~21us without ever blocking the
                # in-order PE queue on the exp stream
                if c == 1:
                    wT_early[0] = new_wT()
                    wT_early[1] = new_wT()
                    for g in range(2):
                        pend.extend(
                            (lambda g=g, st=st: emit_tile(qsrc[g], wT_early[g], 0, st))
                            for st in [16] + list(range(0, 8))
                        )
                elif c >= 2:
                    for g in range(2):
                        pend.extend(
                            (lambda g=g, st=st: emit_tile(qsrc[g], wT_early[g], 0, st))
                            for st in range(4 * c, 4 * c + 4)
                        )
                # deferred q23 matmuls + its rope
                if c > 0:
                    for xtile, k0, kn in xts:
                        for k4 in range(kn):
                            k = k0 + k4
                            nc.tensor.matmul(
                                q23ps, lhsT=wqs(k)[:, 128:256], rhs=xtile[:, k4, :],
                                start=(k == 0), stop=(k == 15),
                            )
                        pump(2)
                nc.vector.tensor_copy(out=q23[:, cs], in_=q23ps)
                rope_chunk(q23, cs, 128)
                nc.gpsimd.dma_start(out=qx3[:, cs], in_=q23[DH:128, cs])
            pump(len(pend))  # drain any leftovers (ACT is behind PE here)
            # phase-2-only constants, placed after the chunk loop so their
            # consumers never head-block the in-order DVE/Pool queues while
            # the projection copies are pending
            nc.vector.tensor_copy(out=v_sb[0:SINK, NST - 1, 0:DH], in_=sinkv_st)
            nc.vector.memset(v_sb[:, :, DH : DH + 1], 1.0)
            make_identity(nc, ident)

        # ---- phase 2+3: attention, then per-512 normalize + output ----
        with (
            tc.tile_pool(name="psC", bufs=4, space="PSUM") as psC,
            tc.tile_pool(name="pLate", bufs=1) as pLate,
        ):
            wo_sb = pLate.tile([128, 2, DMODEL], bf16, tag="wo_sb")
            nc.sync.dma_start(out=wo_sb, in_=wo.rearrange("(a p) n -> p a n", p=128))

            # ctx psum (cps pairs) and y/broadcast psum share one 4-buffer
            # rotation: psA's 4 banks + these 4 fill PSUM exactly
            def scratch_ps():
                return psC.tile([128, 512], f32, tag="ps", name="ps")

            def block_tiles(c, g):
                """Allocate this block's wT and list its emission thunks."""
                wT = new_wT()
                return wT, [
                    (lambda st=st: emit_tile(qsrc[g], wT, c * 1024, st))
                    for st in [16] + list(range(16))
                ]

            def transpose_v(st):
                # v^T -> v_sb through a scores-psum region (f32, double
                # buffered): no serial chain, no extra PSUM banks
                sps = psA.tile([128, 1024], f32, tag="sps", name="sps")
                nc.tensor.transpose(
                    sps[:, 0:DH], vTt[:, st * 128 : (st + 1) * 128], ident
                )
                nc.vector.tensor_copy(out=v_sb[:, st, 0:DH], in_=sps[:, 0:DH])

            # ctx(0,0) consumes v_sb st-by-st in order [16, 0..15]; feeding
            # transpose st at slot st keeps each one ~2 slots ahead of its
            # consumer without a serial aux chain
            tv = [(lambda st=st: transpose_v(st)) for st in range(16)]

            def ctx_out(c, g, u, cps_u):
                t0 = c * 1024 + u * 512
                ctmp = pm.tile([DH + 1, 512], f32, tag="ctmp", bufs=2, name="ctmp")
                # sums row first: it heads the ny chain (recip -> broadcast)
                nc.vector.tensor_copy(
                    out=ctmp[DH : DH + 1, :], in_=cps_u[DH : DH + 1, :]
                )
                nc.gpsimd.dma_start(
                    out=sums_sb[g : g + 1, t0 : t0 + 512],
                    in_=ctmp[DH : DH + 1, :],
                )
                nc.vector.tensor_copy(out=ctmp[0:DH, :], in_=cps_u[0:DH, :])
                dst = ctxT[g // 2]
                if g % 2 == 0:
                    nc.vector.tensor_copy(
                        out=dst[0:DH, t0 : t0 + 512], in_=ctmp[0:DH, :]
                    )
                else:
                    # gpsimd DMA: the only engine that casts (f32->bf16)
                    nc.gpsimd.dma_start(
                        out=dst[DH:128, t0 : t0 + 512], in_=ctmp[0:DH, :]
                    )

            def ctx_block(c, g, wT, feed=(), u_outer=False):
                """ctx for one block, st-outer; feeds one emission tile of a
                later block per st so score bursts never stall the in-order
                PE behind the ACT exp stream.  u_outer runs the two token
                halves sequentially so u=0's sums (heading the next ny
                chain) complete at the halfway point — for the last block."""
                c0 = c * 1024
                fi = list(feed)
                order = [16] + list(range(16))
                if u_outer:
                    for u in range(2):
                        cps_u = scratch_ps()
                        for st in order:
                            p = SINK if st == NST - 1 else 128
                            nc.tensor.matmul(
                                cps_u[0 : DH + 1, :],
                                lhsT=v_sb[0:p, st, :],
                                rhs=wT[0:p, st, u * 512 : (u + 1) * 512],
                                start=(st == order[0]),
                                stop=(st == order[-1]),
                            )
                        ctx_out(c, g, u, cps_u)
                    return
                cps = [scratch_ps() for _ in range(2)]
                for idx, st in enumerate(order):
                    if idx < len(fi):
                        fi[idx]()
                    p = SINK if st == NST - 1 else 128
                    for u in range(2):
                        nc.tensor.matmul(
                            cps[u][0 : DH + 1, :],
                            lhsT=v_sb[0:p, st, :],
                            rhs=wT[0:p, st, u * 512 : (u + 1) * 512],
                            start=(st == order[0]),
                            stop=(st == order[-1]),
                        )
                for u in range(2):
                    ctx_out(c, g, u, cps[u])

            def ny_block(c, u, last, feed=()):
                t0 = c * 1024 + u * 512
                uslc = slice(t0, t0 + 512)
                fi = list(feed)
                # first feeds ahead of the broadcast: PE work that covers the
                # sums -> reciprocal chain latency
                for f in fi[0:2]:
                    f()
                rsums = pLate.tile([GROUP, 512], bf16, tag="rsums", bufs=2, name="rsums")
                with nc.allow_low_precision(reason="denominator reciprocals: bf16 rounding is ~0.2% on a pure scale factor"):
                    nc.vector.reciprocal(rsums, sums_sb[:, uslc])
                # per-j broadcast: head j*2 -> partitions 0-63, head
                # j*2+1 -> 64-127.  Each rbps is fully consumed before the
                # yps allocations below recycle its bank (4-buffer rotation)
                rbps = [scratch_ps() for _ in range(2)]
                for j in range(2):
                    nc.tensor.matmul(
                        rbps[j], lhsT=bsel_sb[:, j * 128 : (j + 1) * 128],
                        rhs=rsums, start=True, stop=True,
                    )
                for j in range(2):
                    nc.vector.tensor_mul(ctxT[j][:, uslc], ctxT[j][:, uslc], rbps[j])

                for tt4 in range(4):  # output projection per 128 tokens
                    for f in (fi[tt4 * 2 + 2 : tt4 * 2 + 4] if tt4 < 3 else []):
                        f()
                    tt = t0 // 128 + tt4
                    y_sb = pLate.tile([128, DMODEL], bf16, tag="y_sb", bufs=2, name="y_sb")
                    for nck in range(4):
                        yps = scratch_ps()
                        for j in range(2):
                            nc.tensor.matmul(
                                yps,
                                lhsT=ctxT[j][:, tt * 128 : (tt + 1) * 128],
                                rhs=wo_sb[:, j, nck * 512 : (nck + 1) * 512],
                                start=(j == 0),
                                stop=(j == 1),
                            )
                        ysl = slice(nck * 512, (nck + 1) * 512)
                        if last and nck % 2 == 1:
                            # ACT is idle once its exp stream ends; share the
                            # tail copies between DVE and ACT
                            nc.scalar.copy(out=y_sb[:, ysl], in_=yps)
                        else:
                            nc.vector.tensor_copy(out=y_sb[:, ysl], in_=yps)
                        if last and tt4 == 3 and nck == 1:
                            # very last tile: DMA the first half early so the
                            # final transfer overlaps the remaining copies
                            nc.sync.dma_start(
                                out=out[tt * 128 : (tt + 1) * 128, 0:1024],
                                in_=y_sb[:, 0:1024],
                            )
                    if last and tt4 == 3:
                        nc.sync.dma_start(
                            out=out[tt * 128 : (tt + 1) * 128, 1024:2048],
                            in_=y_sb[:, 1024:2048],
                        )
                    else:
                        nc.sync.dma_start(out=out[tt * 128 : (tt + 1) * 128, :], in_=y_sb)

            # software pipeline: each ctx/ny interleaves the NEXT block's
            # scores+exp (or a v transpose); wT rotation (bufs=3) frees each
            # buffer exactly one block before reuse.  Every ctx(1,g) precedes
            # ny(1,*) (sums dependency).
            transpose_v(0)
            transpose_v(1)
            ctx_block(0, 0, wT_early[0], feed=tv[2:])
            wT02, t02 = block_tiles(0, 2)
            ctx_block(0, 1, wT_early[1], feed=t02)
            wT03, t03 = block_tiles(0, 3)
            ctx_block(0, 2, wT02, feed=t03)
            wT10, t10 = block_tiles(1, 0)
            ctx_block(0, 3, wT03, feed=t10)
            wT11, t11 = block_tiles(1, 1)
            ny_block(0, 0, False, feed=t11[:8])
            ctx_block(1, 0, wT10, feed=t11[8:])
            wT12, t12 = block_tiles(1, 2)
            ny_block(0, 1, False, feed=t12[:8])
            wT13, t13 = block_tiles(1, 3)
            ctx_block(1, 1, wT11, feed=t12[8:] + t13[:8])
            ctx_block(1, 2, wT12, feed=t13[8:])
            ctx_block(1, 3, wT13, u_outer=True)
            ny_block(1, 0, True)
            ny_block(1, 1, True)

    nc.compile()
    return nc


def _host_inputs(x, kv_cache, Wq, Wk, Wv, Wo, start_pos):
    """Build the 8 per-core input dicts."""
    from ml_dtypes import bfloat16

    f32 = np.float32
    xT = np.ascontiguousarray(np.asarray(x, f32)[0].T.astype(bfloat16))

    inv_freq = (1.0 / (10000.0 ** (np.arange(0, DH, 2, dtype=f32) / DH))).astype(f32)
    pos = np.arange(start_pos, start_pos + T, dtype=f32)
    ang = pos[:, None] * inv_freq[None, :]
    cosT = np.cos(ang).T.astype(f32)  # (32, T)
    sinT = np.sin(ang).T.astype(f32)
    cosb = np.ascontiguousarray(np.concatenate([cosT] * 4, axis=0)).astype(bfloat16)
    sinb = np.ascontiguousarray(
        np.concatenate([-sinT, sinT, -sinT, sinT], axis=0)
    ).astype(bfloat16)

    # 32-row swap within each 64-block: swp[p, i] = 1 iff p = swap(i)
    swp = np.zeros((128, 128), dtype=bfloat16)
    for i in range(128):
        blk = (i // 64) * 64
        swp[blk + ((i - blk) + 32) % 64, i] = 1
    # denominator broadcast selectors (see kernel): one column block per j
    bsel_m = np.zeros((4, 256), dtype=bfloat16)
    for j in range(2):
        for h2 in range(2):
            bsel_m[j * 2 + h2, j * 128 + h2 * 64 : j * 128 + (h2 + 1) * 64] = 1

    Wq = np.asarray(Wq, f32)
    Wk = np.asarray(Wk, f32)
    Wv = np.asarray(Wv, f32)
    Wo = np.asarray(Wo, f32)
    kv_cache = np.asarray(kv_cache, f32)

    in_maps = []
    for i in range(NKV):
        sink = kv_cache[0, i, :SINK, :]
        sink_kT = np.ascontiguousarray(sink.T).astype(bfloat16)
        in_maps.append(
            {
                "xT": xT,
                "wq": np.ascontiguousarray(
                    Wq[:, i * GROUP * DH : (i + 1) * GROUP * DH]
                ).astype(bfloat16),
                "wkv": np.ascontiguousarray(
                    np.concatenate(
                        [Wk[:, i * DH : (i + 1) * DH], Wv[:, i * DH : (i + 1) * DH]],
                        axis=1,
                    )
                ).astype(bfloat16),
                "wo": np.ascontiguousarray(
                    Wo[i * GROUP * DH : (i + 1) * GROUP * DH, :]
                ).astype(bfloat16),
                "sink_kT": sink_kT,
                "sink_v": np.ascontiguousarray(sink).astype(bfloat16),
                "cosb": cosb,
                "sinb": sinb,
                "swp": swp,
                "bsel": bsel_m,
            }
        )
    return in_maps


def run(inputs, trace=False, trace_kwargs=None):
    """Run the 8-core kernel; returns (y, BassKernelResults)."""
    from concourse.bass_utils import run_bass_kernel_spmd

    if "nc" not in _CACHE:
        _CACHE["nc"] = _build_nc()
    nc = _CACHE["nc"]

    start_pos = int(np.asarray(inputs["start_pos"]))
    in_maps = _host_inputs(
        inputs["x"], inputs["kv_cache"], inputs["Wq"], inputs["Wk"], inputs["Wv"],
        inputs["Wo"], start_pos,
    )
    kwargs = {}
    if trace:
        kwargs["trace"] = True
        if trace_kwargs:
            kwargs["trace_kwargs"] = trace_kwargs
    res = run_bass_kernel_spmd(nc, in_maps, core_ids=list(range(NKV)), **kwargs)

    y = res.results[0]["out"].astype(np.float64)
    for i in range(1, NKV):
        y += res.results[i]["out"]
    y = (y + np.asarray(inputs["bo"], np.float64)[None, :]).astype(np.float32)
    return y[None], res


def kernel(**inputs):
    y, _ = run(inputs)
    return y
